# revision 22
# baseline (speedup 1.0000x reference)
"""Trainium2 Bass kernel for nn_CrossAttention (B=4, Lq=Lk=2048, D=1024, H=16, d=64).

Sharding: 8 cores = 4 batches x 2 head-groups (8 heads each).
Each core computes a partial out^T = Wout_g^T @ y_g^T for its (batch, head-group);
host sums the two head-group partials per batch and transposes.

Device layout is feature-major ("T" = [feature, seq]) throughout, all-bf16 on
the PE (fp32 PSUM accumulate):
  qT/kT: [128, L] bf16 per head-pair (2 heads x 64 dims on partitions)
  rotate-half is computed as a second projection with column-permuted weights
  (wqr/wkr) instead of cross-partition copies.
  Both RMSNorm rstds are folded into qT/kT (broadcast matmul), so the exp
  scale is the constant 1/8.
  Scores for the two heads of a pair are issued adjacently at base partitions
  0/64 -> row-tiled concurrent matmuls (K=64 each).
  attnV uses a ones-augmented V (M=65) to produce softmax denominators.
  Phase C is software-pipelined: scores(kc) issue before attnV(kc-1).
"""
import os
import numpy as np
from contextlib import ExitStack

import ml_dtypes

import concourse.bass as bass
import concourse.tile as tile
from concourse import bacc, mybir
from concourse.bass_utils import run_bass_kernel_spmd

F32 = mybir.dt.float32
F32R = mybir.dt.float32r
BF16 = mybir.dt.bfloat16
EXP = mybir.ActivationFunctionType.Exp
SQUARE = mybir.ActivationFunctionType.Square
SQRT = mybir.ActivationFunctionType.Sqrt

D = 1024          # model dim
L = 2048          # seq len (q and k)
HC = 8            # heads per core
DH = 64           # head dim
F = HC * DH       # 512 local features
N_CORES = 8
EPS = float(np.finfo(np.float32).eps)

LAST_RESULTS = None  # BassKernelResults of the most recent run (for test harness)
_NC = None


# --------------------------------------------------------------------------- #
# Device program
# --------------------------------------------------------------------------- #

def _proj_norm_rope(tc, pool, pps, dst, x_sb, w_sb, wr_sb, c_sb, s_sb, bdiag,
                    bmap, lh, eps_t):
    """Project one seq-half of x into feature-major dst [128, L] x4 (bf16),
    with RMSNorm rstd and RoPE folded in.

    dst chunk = ((x@W) * C + (x@Wr) * S) * broadcast(rstd)

    PSUM evictions run on ACT (idle here); DVE combines are all-bf16 SBUF
    so they hit the 2x perf mode.
    """
    nc = tc.nc

    for fb in range(4):           # head pair
        psq, psr, sqs = [], [], []
        vps = pps.tile([2, 1024], F32, tag="vps", bufs=1)
        for qn in range(2):       # 512-wide seq chunk within the half
            pq = pps.tile([128, 512], F32, tag="psq", bufs=2)
            for dc in range(8):
                nc.tensor.matmul(
                    pq[:],
                    w_sb[dc][:, fb * 128:(fb + 1) * 128],
                    x_sb[dc][:, qn * 512:(qn + 1) * 512],
                    start=(dc == 0), stop=(dc == 7))
            pr = pps.tile([128, 512], F32, tag="psr", bufs=2)
            for dc in range(8):
                nc.tensor.matmul(
                    pr[:],
                    wr_sb[dc][:, fb * 128:(fb + 1) * 128],
                    x_sb[dc][:, qn * 512:(qn + 1) * 512],
                    start=(dc == 0), stop=(dc == 7))
            sq = pool.tile([128, 512], BF16, tag="sq", bufs=2)
            nc.scalar.activation(sq[:], pq[:], SQUARE)
            nc.tensor.matmul(vps[:, qn * 512:(qn + 1) * 512], bdiag, sq[:],
                             start=True, stop=True)
            psq.append(pq)
            psr.append(pr)
        # rstd for the whole 1024-wide half of this head pair
        std = pool.tile([2, 1024], F32, tag="std", bufs=1)
        nc.scalar.activation(std[:], vps[:], SQRT,
                             bias=eps_t[:], scale=1.0 / 64.0)
        rstd = pool.tile([2, 1024], F32, tag="rstd", bufs=1)
        nc.vector.reciprocal_approx_fast(out=rstd[:], in_=std[:])
        rstd_h = pool.tile([2, 1024], BF16, tag="rstd_h", bufs=1)
        nc.vector.tensor_copy(rstd_h[:], rstd[:])
        for qn in range(2):
            col0 = lh * 1024 + qn * 512
            bps = pps.tile([128, 512], F32, tag="bps", bufs=1)
            nc.tensor.matmul(bps[:], bmap,
                             rstd_h[:, qn * 512:(qn + 1) * 512],
                             start=True, stop=True)
            # evict PSUM -> bf16 SBUF on ACT
            qsb = pool.tile([128, 512], BF16, tag="qsb", bufs=2)
            nc.scalar.copy(qsb[:], psq[qn][:])
            rsb = pool.tile([128, 512], BF16, tag="rsb", bufs=2)
            nc.scalar.copy(rsb[:], psr[qn][:])
            bsb = pool.tile([128, 512], BF16, tag="bsb", bufs=2)
            nc.scalar.copy(bsb[:], bps[:])
            # rope combine + norm on DVE (all-bf16 SBUF -> 2x mode)
            t1 = pool.tile([128, 512], BF16, tag="t1", bufs=2)
            nc.vector.tensor_mul(t1[:], qsb[:], c_sb[:, col0:col0 + 512])
            t2 = pool.tile([128, 512], BF16, tag="t2", bufs=2)
            nc.vector.tensor_mul(t2[:], rsb[:], s_sb[:, col0:col0 + 512])
            t3 = pool.tile([128, 512], BF16, tag="t3", bufs=2)
            nc.vector.tensor_add(t3[:], t1[:], t2[:])
            nc.vector.tensor_mul(dst[fb][:, col0:col0 + 512], t3[:], bsb[:])


def _build_program():
    nc = bacc.Bacc("TRN2", target_bir_lowering=False, debug=False,
                   num_devices=N_CORES)
    dt = nc.dram_tensor
    xqT = dt("xqT", (D, L), BF16, kind="ExternalInput").ap()
    xkvT = dt("xkvT", (D, L), BF16, kind="ExternalInput").ap()
    wq = dt("wq", (D, F), BF16, kind="ExternalInput").ap()
    wqr = dt("wqr", (D, F), BF16, kind="ExternalInput").ap()
    wk = dt("wk", (D, F), BF16, kind="ExternalInput").ap()
    wkr = dt("wkr", (D, F), BF16, kind="ExternalInput").ap()
    wv = dt("wv", (D, F), BF16, kind="ExternalInput").ap()
    wout = dt("wout", (F, D), BF16, kind="ExternalInput").ap()
    cq = dt("cq", (128, L), BF16, kind="ExternalInput").ap()
    sq_t = dt("sq", (128, L), BF16, kind="ExternalInput").ap()
    ck = dt("ck", (128, L), BF16, kind="ExternalInput").ap()
    sk_t = dt("sk", (128, L), BF16, kind="ExternalInput").ap()
    bdiag_d = dt("bdiag", (128, 2), BF16, kind="ExternalInput").ap()
    bmap_d = dt("bmap", (2, 128), BF16, kind="ExternalInput").ap()
    selA_d = dt("selA", (128, 128), BF16, kind="ExternalInput").ap()
    selB_d = dt("selB", (128, 128), BF16, kind="ExternalInput").ap()
    outT = dt("outT", (D, L), F32, kind="ExternalOutput").ap()

    with tile.TileContext(nc) as tc:
        with ExitStack() as ctx:
            big = ctx.enter_context(tc.tile_pool(name="big", bufs=1))

            kT = [big.tile([128, L], BF16, tag=f"kT{i}", name=f"kT{i}")
                  for i in range(4)]
            qT = [big.tile([128, L], BF16, tag=f"qT{i}", name=f"qT{i}")
                  for i in range(4)]
            vaug = [big.tile([128, HC * 65], BF16, tag=f"v{i}", name=f"vaug{i}")
                    for i in range(16)]
            ytr = [big.tile([128, L], BF16, tag=f"ytr{i}", name=f"ytr{i}")
                   for i in range(4)]
            sums_g = [big.tile([128, L], F32, tag=f"sums{g}", name=f"sums{g}")
                      for g in range(2)]

            bdiag = big.tile([128, 2], BF16, tag="bdiag")
            nc.sync.dma_start(bdiag[:], bdiag_d[:])
            bmap = big.tile([2, 128], BF16, tag="bmap")
            nc.sync.dma_start(bmap[:], bmap_d[:])
            eps_t = big.tile([2, 1], F32, tag="eps")
            nc.gpsimd.memset(eps_t[:], EPS)
            for g in range(2):
                nc.gpsimd.memset(sums_g[g][:], 1.0)
            for i in range(16):
                nc.gpsimd.memset(vaug[i][:], 1.0)

            # ---- x prefetch for both projection phases (issued upfront) ----
            xio = ctx.enter_context(tc.tile_pool(name="xio", bufs=1))
            xa_sb, xb_sb = [], []
            for lh in range(2):
                row = []
                for dc in range(8):
                    x = xio.tile([128, 1024], BF16, tag=f"xa{lh}_{dc}",
                                 name=f"xa{lh}_{dc}")
                    nc.sync.dma_start(
                        x[:], xkvT[dc * 128:(dc + 1) * 128,
                                   lh * 1024:(lh + 1) * 1024])
                    row.append(x)
                xa_sb.append(row)
            for lh in range(2):
                row = []
                for dc in range(8):
                    x = xio.tile([128, 1024], BF16, tag=f"xb{lh}_{dc}",
                                 name=f"xb{lh}_{dc}")
                    nc.sync.dma_start(
                        x[:], xqT[dc * 128:(dc + 1) * 128,
                                  lh * 1024:(lh + 1) * 1024])
                    row.append(x)
                xb_sb.append(row)

            # ---- Phase A: kv projections (k feature-major + v into vaug) ----
            with ExitStack() as actx:
                apool = actx.enter_context(tc.tile_pool(name="a_sb", bufs=1))
                aps = actx.enter_context(
                    tc.tile_pool(name="a_ps", bufs=1, space="PSUM"))
                c_sb = apool.tile([128, L], BF16, tag="ctab")
                nc.sync.dma_start(c_sb[:], ck[:])
                s_sb = apool.tile([128, L], BF16, tag="stab")
                nc.sync.dma_start(s_sb[:], sk_t[:])
                wk_sb, wkr_sb, wv_sb = [], [], []
                for dc in range(8):
                    w = apool.tile([128, F], BF16, tag=f"wk{dc}",
                                   name=f"wk_sb{dc}")
                    nc.sync.dma_start(w[:], wk[dc * 128:(dc + 1) * 128, :])
                    wk_sb.append(w)
                    w = apool.tile([128, F], BF16, tag=f"wkr{dc}",
                                   name=f"wkr_sb{dc}")
                    nc.sync.dma_start(w[:], wkr[dc * 128:(dc + 1) * 128, :])
                    wkr_sb.append(w)
                    w = apool.tile([128, F], BF16, tag=f"wv{dc}",
                                   name=f"wv_sb{dc}")
                    nc.sync.dma_start(w[:], wv[dc * 128:(dc + 1) * 128, :])
                    wv_sb.append(w)

                for lh in range(2):
                    x_sb = xa_sb[lh]
                    _proj_norm_rope(tc, apool, aps, kT, x_sb, wk_sb, wkr_sb,
                                    c_sb, s_sb, bdiag[:], bmap[:], lh, eps_t)
                    for lc in range(8):
                        kc = lh * 8 + lc
                        ps = aps.tile([128, 512], F32, tag="vproj", bufs=1)
                        for dc in range(8):
                            nc.tensor.matmul(
                                ps[:],
                                x_sb[dc][:, lc * 128:(lc + 1) * 128],
                                wv_sb[dc][:],
                                start=(dc == 0), stop=(dc == 7))
                        va3 = vaug[kc].rearrange("p (h c) -> p h c", c=65)
                        ps3 = ps.rearrange("p (h c) -> p h c", c=64)
                        nc.vector.tensor_copy(va3[:, :, 0:64], ps3[:])

            # ---- Phase B: q projection ----
            with ExitStack() as bctx:
                bpool = bctx.enter_context(tc.tile_pool(name="b_sb", bufs=1))
                c_sb = bpool.tile([128, L], BF16, tag="ctab")
                nc.sync.dma_start(c_sb[:], cq[:])
                s_sb = bpool.tile([128, L], BF16, tag="stab")
                nc.sync.dma_start(s_sb[:], sq_t[:])
                wq_sb, wqr_sb = [], []
                for dc in range(8):
                    w = bpool.tile([128, F], BF16, tag=f"wq{dc}",
                                   name=f"wq_sb{dc}")
                    nc.sync.dma_start(w[:], wq[dc * 128:(dc + 1) * 128, :])
                    wq_sb.append(w)
                    w = bpool.tile([128, F], BF16, tag=f"wqr{dc}",
                                   name=f"wqr_sb{dc}")
                    nc.sync.dma_start(w[:], wqr[dc * 128:(dc + 1) * 128, :])
                    wqr_sb.append(w)
                bps_pool = bctx.enter_context(
                    tc.tile_pool(name="b_ps", bufs=1, space="PSUM"))
                for lh in range(2):
                    _proj_norm_rope(tc, bpool, bps_pool, qT, xb_sb[lh],
                                    wq_sb, wqr_sb, c_sb, s_sb, bdiag[:],
                                    bmap[:], lh, eps_t)

            # ---- Phase D prefetch (overlaps attention) ----
            dpool = ctx.enter_context(tc.tile_pool(name="out_sb", bufs=1))
            wo_sb = []
            for fc in range(4):
                w = dpool.tile([128, D], BF16, tag=f"wo{fc}",
                               name=f"wo_sb{fc}")
                nc.sync.dma_start(w[:], wout[fc * 128:(fc + 1) * 128, :])
                wo_sb.append(w)
            sel_sb = []
            for i, sd in enumerate((selA_d, selB_d)):
                s = dpool.tile([128, 128], BF16, tag=f"sel{i}",
                               name=f"sel{i}")
                nc.sync.dma_start(s[:], sd[:])
                sel_sb.append(s)

            # ---- Phase C: attention ----
            with ExitStack() as cctx:
                cpool = cctx.enter_context(tc.tile_pool(name="att_sb", bufs=1))
                cps = cctx.enter_context(
                    tc.tile_pool(name="att_ps", bufs=1, space="PSUM"))
                def attn_v(yA, yB, hA, hB, ptA, ptB, pkc):
                    va3 = vaug[pkc].rearrange("p (h c) -> p h c", c=65)
                    for j in range(2):
                        nc.tensor.matmul(
                            yA[:, j * 512:(j + 1) * 512],
                            va3[:, hA, :],
                            ptA[:, j * 512:(j + 1) * 512],
                            start=(pkc == 0), stop=(pkc == 15))
                        nc.tensor.matmul(
                            yB[:, j * 512:(j + 1) * 512],
                            va3[:, hB, :],
                            ptB[:, j * 512:(j + 1) * 512],
                            start=(pkc == 0), stop=(pkc == 15))

                for p in range(4):
                    hA, hB = 2 * p, 2 * p + 1
                    for qh in range(2):
                        yA = cps.tile([65, 1024], F32, tag="yA", bufs=1,
                                      name=f"yA{p}_{qh}")
                        yB = cps.tile([65, 1024], F32, tag="yB", bufs=1,
                                      name=f"yB{p}_{qh}")
                        pend = None
                        for kc in range(16):
                            spsA = cps.tile([128, 1024], F32, tag="spsA",
                                            bufs=1)
                            spsB = cps.tile([128, 1024], F32, tag="spsB",
                                            bufs=1)
                            for j in range(2):
                                q0 = qh * 1024 + j * 512
                                # adjacent pair: row-tiles (0,0) and (64,0)
                                nc.tensor.matmul(
                                    spsA[:, j * 512:(j + 1) * 512],
                                    kT[p][0:64, kc * 128:(kc + 1) * 128],
                                    qT[p][0:64, q0:q0 + 512],
                                    start=True, stop=True)
                                nc.tensor.matmul(
                                    spsB[:, j * 512:(j + 1) * 512],
                                    kT[p][64:128, kc * 128:(kc + 1) * 128],
                                    qT[p][64:128, q0:q0 + 512],
                                    start=True, stop=True)
                            ptA = cpool.tile([128, 1024], BF16, tag="ptA",
                                             bufs=2)
                            nc.scalar.activation(ptA[:], spsA[:], EXP,
                                                 scale=0.125)
                            ptB = cpool.tile([128, 1024], BF16, tag="ptB",
                                             bufs=2)
                            nc.scalar.activation(ptB[:], spsB[:], EXP,
                                                 scale=0.125)
                            if pend is not None:
                                attn_v(yA, yB, hA, hB, *pend)
                            pend = (ptA, ptB, kc)
                        attn_v(yA, yB, hA, hB, *pend)
                        # evict y + softmax sums
                        c0 = qh * 1024
                        nc.vector.tensor_copy(
                            ytr[p][0:64, c0:c0 + 1024], yA[0:64, :])
                        nc.vector.tensor_copy(
                            ytr[p][64:128, c0:c0 + 1024], yB[0:64, :])
                        nc.vector.tensor_copy(
                            sums_g[hA // 4][32 * (hA % 4):32 * (hA % 4) + 1,
                                            c0:c0 + 1024], yA[64:65, :])
                        nc.vector.tensor_copy(
                            sums_g[hB // 4][32 * (hB % 4):32 * (hB % 4) + 1,
                                            c0:c0 + 1024], yB[64:65, :])

            # ---- Phase D: normalize + output projection ----
            with ExitStack() as dctx:
                dps = dctx.enter_context(
                    tc.tile_pool(name="out_ps", bufs=1, space="PSUM"))
                rs_g = []
                for g in range(2):
                    rs32 = dpool.tile([128, L], F32, tag="rs32", bufs=2,
                                      name=f"rs32_{g}")
                    nc.vector.reciprocal_approx_fast(
                        out=rs32[:], in_=sums_g[g][:])
                    rs = dpool.tile([128, L], BF16, tag="rs", bufs=2,
                                    name=f"rs{g}")
                    nc.vector.tensor_copy(rs[:], rs32[:])
                    rs_g.append(rs)
                for fb in range(4):
                    sel = sel_sb[fb % 2]
                    bps = dps.tile([128, L], F32, tag="bc2", bufs=1)
                    for qn in range(4):
                        nc.tensor.matmul(
                            bps[:, qn * 512:(qn + 1) * 512],
                            sel[:],
                            rs_g[fb // 2][:, qn * 512:(qn + 1) * 512],
                            start=True, stop=True)
                    nc.vector.tensor_mul(ytr[fb][:], ytr[fb][:], bps[:])
                for nb in range(8):
                    for qn in range(4):
                        ps = dps.tile([128, 512], F32, tag="oproj", bufs=3)
                        for fc in range(4):
                            nc.tensor.matmul(
                                ps[:],
                                wo_sb[fc][:, nb * 128:(nb + 1) * 128],
                                ytr[fc][:, qn * 512:(qn + 1) * 512],
                                start=(fc == 0), stop=(fc == 3))
                        ot = dpool.tile([128, 512], F32, tag="ot", bufs=3)
                        nc.scalar.copy(ot[:], ps[:])
                        nc.sync.dma_start(
                            outT[nb * 128:(nb + 1) * 128,
                                 qn * 512:(qn + 1) * 512], ot[:])
    nc.compile()
    return nc


def get_nc():
    global _NC
    if _NC is None:
        _NC = _build_program()
    return _NC


# --------------------------------------------------------------------------- #
# Host side
# --------------------------------------------------------------------------- #

def _rope_tables(pos, g):
    """Feature-major folded RoPE(+gain) tables, replicated for a 2-head tile."""
    pos = np.asarray(pos).astype(np.float32)
    g = np.asarray(g, dtype=np.float32)
    inv = (1.0 / (10000.0 ** (np.arange(0, DH, 2, dtype=np.float32)
                              / np.float32(DH)))).astype(np.float32)
    ang = pos[:, None] * inv[None, :]                      # (L, 32)
    cos, sin = np.cos(ang, dtype=np.float32), np.sin(ang, dtype=np.float32)
    j = np.arange(DH)
    C = (g[j][:, None] * cos[:, j % 32].T).astype(np.float32)       # (64, L)
    sign = np.where(j < 32, -1.0, 1.0).astype(np.float32)
    S = (sign[:, None] * g[(j + 32) % 64][:, None]
         * sin[:, j % 32].T).astype(np.float32)
    return (np.ascontiguousarray(np.tile(C, (2, 1))),
            np.ascontiguousarray(np.tile(S, (2, 1))))     # (128, L) each


def _swap_halves(w):
    """Swap the 32-col halves of each head's 64-col block (rotate-half)."""
    w3 = w.reshape(w.shape[0], -1, 2, 32)
    return np.ascontiguousarray(w3[:, :, ::-1, :].reshape(w.shape))


def make_in_maps(queries, kv, Wq, Wkv, Wout, g_q, g_k, pos_q, pos_k):
    bf16 = ml_dtypes.bfloat16
    queries = np.asarray(queries, dtype=np.float32)
    kv = np.asarray(kv, dtype=np.float32)
    Wq = np.asarray(Wq, dtype=np.float32)
    Wkv = np.asarray(Wkv, dtype=np.float32)
    Wout = np.asarray(Wout, dtype=np.float32)

    cq, sq = _rope_tables(pos_q, g_q)
    ck, sk = _rope_tables(pos_k, g_k)
    bdiag = np.zeros((128, 2), bf16)
    bdiag[0:64, 0] = 1.0
    bdiag[64:128, 1] = 1.0
    bmap = np.zeros((2, 128), bf16)
    bmap[0, 0:64] = 1.0
    bmap[1, 64:128] = 1.0
    selA = np.zeros((128, 128), bf16)
    selA[0, 0:64] = 1.0
    selA[32, 64:128] = 1.0
    selB = np.zeros((128, 128), bf16)
    selB[64, 0:64] = 1.0
    selB[96, 64:128] = 1.0

    Wkv3 = Wkv.reshape(D, 16, 2 * DH)
    in_maps = []
    for c in range(N_CORES):
        b, grp = c // 2, c % 2
        hs = slice(grp * HC, (grp + 1) * HC)
        wq_g = Wq[:, grp * F:(grp + 1) * F]
        wk_g = Wkv3[:, hs, :DH].reshape(D, F)
        in_maps.append({
            "xqT": np.ascontiguousarray(queries[b].T).astype(bf16),
            "xkvT": np.ascontiguousarray(kv[b].T).astype(bf16),
            "wq": np.ascontiguousarray(wq_g).astype(bf16),
            "wqr": _swap_halves(wq_g).astype(bf16),
            "wk": np.ascontiguousarray(wk_g).astype(bf16),
            "wkr": _swap_halves(wk_g).astype(bf16),
            "wv": np.ascontiguousarray(
                Wkv3[:, hs, DH:].reshape(D, F)).astype(bf16),
            "wout": np.ascontiguousarray(
                Wout[grp * F:(grp + 1) * F, :]).astype(bf16),
            "cq": cq.astype(bf16), "sq": sq.astype(bf16),
            "ck": ck.astype(bf16), "sk": sk.astype(bf16),
            "bdiag": bdiag, "bmap": bmap, "selA": selA, "selB": selB,
        })
    return in_maps


def kernel(queries, kv, Wq, Wkv, Wout, g_q, g_k, pos_q, pos_k):
    global LAST_RESULTS
    nc = get_nc()
    in_maps = make_in_maps(queries, kv, Wq, Wkv, Wout, g_q, g_k, pos_q, pos_k)
    trace = bool(int(os.environ.get("KERNEL_TRACE", "0")))
    kw = {}
    if trace:
        kw["tmpdir"] = os.environ.get("KERNEL_TRACE_DIR") or None
    res = run_bass_kernel_spmd(nc, in_maps, core_ids=list(range(N_CORES)),
                               trace=trace, **kw)
    LAST_RESULTS = res
    out = np.empty((4, L, D), np.float32)
    for b in range(4):
        out[b] = (res.results[2 * b]["outT"]
                  + res.results[2 * b + 1]["outT"]).T
    return out
